# revision 15
# baseline (speedup 1.0000x reference)
"""LIF spiking-neuron recurrence on Trainium2 (8 NeuronCores).

Reference semantics (TAU=1, THRESH=1, f32):
    mem = 0
    for t in range(T):
        mem = mem + x[t]
        spike[t] = (mem >= 1.0) ? 1.0 : 0.0
        mem = mem * (1 - spike[t])        # hard reset

Sharding: data-parallel over the batch axis (B=128 -> 16 rows/core).
Per-core layout: the [T, 16, 16384] shard is viewed as [T, 128, 2048]
(partition-major within a timestep slab) and pre-transposed on the host
to [128, T, 2048] so each partition's DMA runs are contiguous.

Engine mapping per timestep (tile [128, 2048] f32):
    DVE : v = mem + x_t              (tensor_tensor add, 1x, ~2.29us)
    ACT : s8 = Sign(v - 1)           (ONE pass, ~2.0us; exact -1/0/+1,
                                      probed on HW incl. the v==1 tie
                                      -> 0 and +-1ulp neighbours)
    DVE : mem = (v < 1) * v          (scalar_tensor_tensor, 1x, ~2.29us)
Host maps spike = (s8 >= 0): v<1 -> -1 -> 0; v>=1 -> 0/+1 -> 1. Exact.

Why this shape (probed on this HW, traces in session notes):
  - DVE f32 two-tensor ops run at 1 elem/lane/cycle (~1.12ns/col); the
    2x/4x perf modes only apply to one-tensor-operand ops, so the
    add+reset chain is a hard 2-pass DVE floor (~289us busy).
  - GpSimd add/mult offload was tried and REGRESSES: Pool-engine SBUF
    port contention inflates concurrent DVE ops ~40%, and gp itself
    runs ~4.3-4.9ns/col on strided slices => net loss (405us measured).
  - PE identity-matmul accumulation can't help: fp32r truncates
    operands to FP22 (lossy), true fp32 is 4 cycles/row.
  - Sign (1 ACT pass) replaces the baseline's Sqrt+Is_finite (2 passes)
    halving ACT busy and shortening the per-step tail; int8 spike
    stores keep DMA (~80MB/core) well under the ~370GB/s DMA ceiling.

DMAs are HWDGE (loads on SP ring, stores on ACT ring); loads are
per-step 1MB transfers with 6 prefetch buffers (splitting them into
smaller chunks regresses: 1KB descriptors clog the ring), stores
per-4-step 1MB int8 transfers except the last group which stores
per-step, with the final step half-split so its store overlaps its
Sign. A warmup Sign on a [P,1] tile pulls the 1.28us ACT table load
into the DGE-init dead time. Step 0 uses x_0 directly (mem starts at
0); the final step's reset is dead code.

Measured on 8 axon-tunneled trn2 cores: 316624 ns HW exec time
(neuron-profile, core 0; +-2us run-to-run), bit-exact vs the jax f32
reference (baseline Sqrt/Is_finite variant: 321301 ns). Breakdown:
~289us DVE busy (the hard floor), ~8.4us DVE dispatch gaps (70ns/instr
sequencer floor), ~10us framework preamble + cold first load (pipeline
start is paced by the x_1 load arrival), ~5us drain.
"""

import numpy as np

try:
    import concourse  # noqa: F401
except ImportError:  # pragma: no cover
    import sys

    for _p in ("/opt/trn_rl_repo", "/root/.axon_site/_ro/trn_rl_repo"):
        if _p not in sys.path:
            sys.path.insert(0, _p)

from concourse import bacc, mybir
from concourse.bass_utils import run_bass_kernel_spmd
from concourse.mybir import ActivationFunctionType as AF
from concourse.mybir import AluOpType
from concourse.tile import TileContext

T, B, D = 64, 128, 16384
NCORES = 8
BL = B // NCORES  # 16 batch rows per core
P = 128  # SBUF partitions
F = (BL * D) // P  # 2048 free elements per timestep slab
SG = 4  # timesteps per store group


def build_nc(t_steps=T, f_free=F, sg=SG, x_bufs=6, v_bufs=3, s_bufs=3):
    """Build + compile the per-core Bass program (identical on all cores)."""
    f32 = mybir.dt.float32
    s8 = mybir.dt.int8
    nc = bacc.Bacc(
        "TRN2", target_bir_lowering=False, debug=False, num_devices=NCORES
    )
    x_ext = nc.dram_tensor("x", [P, t_steps, f_free], f32, kind="ExternalInput")
    out_ext = nc.dram_tensor(
        "out", [P, t_steps, f_free], s8, kind="ExternalOutput"
    )
    n_groups = t_steps // sg
    with TileContext(nc) as tc:
        with (
            tc.tile_pool(name="xp", bufs=x_bufs) as xp,
            tc.tile_pool(name="vp", bufs=v_bufs) as vp,
            tc.tile_pool(name="sp", bufs=s_bufs) as sp,
            tc.tile_pool(name="mp", bufs=1) as mp,
        ):
            mem = mp.tile([P, f_free], f32)
            bm1 = mp.tile([P, 1], f32, name="bm1")
            nc.vector.memset(bm1[:], -1.0)
            # tiny warmup Sign: pulls the ACT table load into the DGE-init
            # dead time so step 0's first real Sign doesn't pay the 1.28us
            warm = mp.tile([P, 1], s8, name="warm")
            nc.scalar.activation(warm[:], bm1[:], AF.Sign, bias=0.0, scale=1.0)
            spk = None
            for t in range(t_steps):
                g, j = divmod(t, sg)
                xt = xp.tile([P, f_free], f32, name="xt")
                if t == 0:
                    # halve the cold first load across BOTH idle rings so
                    # descriptor fetch parallelizes; compute chases halves
                    # (extending this to t=1,2 regresses ~6us: the scalar-
                    # ring halves disrupt the steady sync-ring load stream)
                    h = f_free // 2
                    nc.sync.dma_start(xt[:, :h], x_ext[:, 0, :h])
                    nc.scalar.dma_start(xt[:, h:], x_ext[:, 0, h:])
                else:
                    nc.sync.dma_start(xt[:], x_ext[:, t, :])
                if j == 0:
                    spk = sp.tile([P, sg * f_free], s8, name="spk")
                ss = spk[:, j * f_free : (j + 1) * f_free]
                if t == 0:
                    # mem==0: pre-reset membrane is just x_0
                    h = f_free // 2
                    for q in range(2):
                        sl = slice(q * h, (q + 1) * h)
                        nc.vector.scalar_tensor_tensor(
                            mem[:, sl], xt[:, sl], 1.0, xt[:, sl],
                            AluOpType.is_lt, AluOpType.mult,
                        )
                        nc.scalar.activation(
                            ss[:, sl], xt[:, sl], AF.Sign, bias=bm1[:],
                            scale=1.0,
                        )
                    continue
                if t == t_steps - 1:
                    # quarter-split the last step so stores overlap the
                    # remaining Signs; reset is dead code
                    q4 = f_free // 4
                    v = vp.tile([P, f_free], f32, name="v")
                    for q in range(4):
                        sl = slice(q * q4, (q + 1) * q4)
                        nc.vector.tensor_tensor(
                            v[:, sl], mem[:, sl], xt[:, sl], AluOpType.add
                        )
                        nc.scalar.activation(
                            ss[:, sl], v[:, sl], AF.Sign, bias=bm1[:],
                            scale=1.0,
                        )
                        nc.scalar.dma_start(out_ext[:, t, sl], ss[:, sl])
                    continue
                v = vp.tile([P, f_free], f32, name="v")
                nc.vector.tensor_tensor(v[:], mem[:], xt[:], AluOpType.add)
                # spike encoding: Sign(v-1) in {-1,0,+1}; host: spike = s8>=0
                nc.scalar.activation(ss, v[:], AF.Sign, bias=bm1[:], scale=1.0)
                nc.vector.scalar_tensor_tensor(
                    mem[:], v[:], 1.0, v[:],
                    AluOpType.is_lt, AluOpType.mult,
                )
                if g == n_groups - 1:
                    # per-step stores so the tail drains quickly
                    nc.scalar.dma_start(out_ext[:, t, :], ss)
                elif j == sg - 1:
                    nc.scalar.dma_start(
                        out_ext[:, g * sg : (g + 1) * sg, :].rearrange(
                            "p t f -> p (t f)"
                        ),
                        spk[:],
                    )
    nc.compile()
    return nc


_cached_nc = None


def _get_nc():
    global _cached_nc
    if _cached_nc is None:
        _cached_nc = build_nc()
    return _cached_nc


def _shard(x):
    """Full [T, B, D] -> list of per-core [P, T, F] contiguous arrays."""
    in_maps = []
    for c in range(NCORES):
        xc = x[:, c * BL : (c + 1) * BL, :].reshape(T, P, F).transpose(1, 0, 2)
        in_maps.append({"x": np.ascontiguousarray(xc)})
    return in_maps


def _gather(results):
    """Per-core [P, T, F] int8 sign outputs -> full [T, B, D] f32 spikes."""
    outs = [
        (np.asarray(results[c]["out"]) >= 0)
        .astype(np.float32)
        .transpose(1, 0, 2)
        .reshape(T, BL, D)
        for c in range(NCORES)
    ]
    return np.concatenate(outs, axis=1)


def run(x, trace=False, **kw):
    """Run on the 8 NeuronCores; returns (output, BassKernelResults)."""
    x = np.ascontiguousarray(np.asarray(x, dtype=np.float32))
    assert x.shape == (T, B, D), x.shape
    nc = _get_nc()
    res = run_bass_kernel_spmd(
        nc, _shard(x), core_ids=list(range(NCORES)), trace=trace, **kw
    )
    return _gather(res.results), res


def kernel(x: np.ndarray) -> np.ndarray:
    out, _ = run(x)
    return out
